# revision 14
# baseline (speedup 1.0000x reference)
"""Trainium2 Bass kernel for HNet attention (B=4, S=2048, H=768, 12 heads, RoPE, causal).

Sharding: 8 cores = 4 batches x 2 head-groups (6 heads each).
Wq/Wk/Wv split column-wise (head axis), Wo row-wise; host sums the two
partial o_proj outputs per batch (the "all-reduce" done at gather time).

Per-core dataflow (v4 — fp16 inputs, natural-layout PV, packed DMAs):
  xT [768,2048] fp16 (host-packed by column-chunk) --PE--> Q,K,V natural
  RoPE on Q,K in natural layout (DVE t1 / Pool t2), PE-transpose fp16
  scoresT[k,q] = kT.T @ qT per (head-pair, par) with PE row groups; causal
    mask folded into the PE as an accumulate-matmul of a -30000 triangle
  exp on ScalarE -> ex fp16; PV natural: po[q, 65] += ex.T @ [V_h | 1]
  (col 64 = softmax sums); normalize via reciprocal + per-head tensor_scalar;
  deferred fill: PE-transpose -> aoT, o_proj fin = aoT.T @ woT, strip store.
"""

import os
import sys

import numpy as np

sys.path.insert(0, "/opt/trn_rl_repo")

from collections import deque
from contextlib import ExitStack

import concourse.bacc as bacc
import concourse.tile as tile
from concourse import mybir
from concourse.bass_utils import run_bass_kernel_spmd

S = 2048
HID = 768
NH = 6            # heads per core
D = 64
F = NH * D        # 384 per-core feature slice
P = 128
SC = S // P       # 16 s-tiles
FC = HID // P     # 6 contraction chunks
MC = F // P       # 3 head-pair chunks
QW = 512          # q strip width
NQ = S // QW      # 4 strips
N_CORES = 8
ROPE_THETA = 10000.0
NEG = -30000.0

F32 = mybir.dt.float32
F16 = mybir.dt.float16
F8 = mybir.dt.float8e4
AF = mybir.ActivationFunctionType

XW = NQ * FC * QW      # packed xT width 12288
WW = FC * 3 * F        # packed wqkv width 6912
CW = SC * 2 * F        # packed cos|sin width 12288


def build_program():
    nc = bacc.Bacc("TRN2", target_bir_lowering=False, debug=False,
                   num_devices=N_CORES)

    xTp_d = nc.dram_tensor("xTp", [P, XW], F16, kind="ExternalInput").ap()
    wp_d = nc.dram_tensor("wp", [P, WW], F16, kind="ExternalInput").ap()
    woT_d = nc.dram_tensor("woT", [P, MC * HID], F16, kind="ExternalInput").ap()
    csn_d = nc.dram_tensor("csn", [P, CW], F16, kind="ExternalInput").ap()
    tri_d = nc.dram_tensor("trineg", [P, P], F16, kind="ExternalInput").ap()
    eye_d = nc.dram_tensor("eye", [P, P], F16, kind="ExternalInput").ap()
    out_d = nc.dram_tensor("out", [S, HID], F32, kind="ExternalOutput").ap()

    with tile.TileContext(nc) as tc, ExitStack() as ctx:
        const_pool = ctx.enter_context(tc.tile_pool(name="const", bufs=1))
        eye_sb = const_pool.tile([P, P], F16, tag="eye")
        nc.sync.dma_start(eye_sb[:], eye_d[:])
        tri_sb = const_pool.tile([P, P], F16, tag="tri")
        nc.sync.dma_start(tri_sb[:], tri_d[:])

        # ---- persistent SBUF; DMA order feeds the prologue first ----
        xw_pool = ctx.enter_context(tc.tile_pool(name="xw", bufs=1))
        wp = xw_pool.tile([P, WW], F16, tag="wp")
        xTp = xw_pool.tile([P, XW], F16, tag="xTp")
        csn = xw_pool.tile([P, CW], F16, tag="csn")
        woT = xw_pool.tile([P, MC * HID], F16, tag="woT")
        for f in range(FC):
            c0 = f * 3 * F
            nc.sync.dma_start(wp[:, c0:c0 + 3 * F], wp_d[:, c0:c0 + 3 * F])
        nc.sync.dma_start(xTp[:, 0:XW // 4], xTp_d[:, 0:XW // 4])
        nc.sync.dma_start(csn[:, 0:CW // 4], csn_d[:, 0:CW // 4])
        nc.sync.dma_start(woT[:], woT_d[:])

        def wslice(kind, f):  # 0=q 1=k 2=v
            c0 = f * 3 * F + kind * F
            return wp[:, c0:c0 + F]

        def xslice(f, s):
            c0 = (s // 4) * (FC * QW) + f * QW + (s % 4) * P
            return xTp[:, c0:c0 + P]

        kT_pool = ctx.enter_context(tc.tile_pool(name="kTp", bufs=1))
        kTs = kT_pool.tile([P, MC * S], F16, tag="kTs")
        v_pool = ctx.enter_context(tc.tile_pool(name="vp", bufs=1))
        vo = [v_pool.tile([P, NH * 65], F16, tag=f"v{s}", name=f"v{s}")
              for s in range(SC)]
        for s in range(SC):
            v3 = vo[s].rearrange("p (h e) -> p h e", h=NH)
            nc.gpsimd.memset(v3[:, :, 64], 1.0)

        with tc.tile_pool(name="rp", bufs=2) as rp_pool, \
             tc.tile_pool(name="qr", bufs=4) as qr_pool, \
             tc.tile_pool(name="qTs", bufs=2) as qTs_pool, \
             tc.tile_pool(name="ao", bufs=3) as ao_pool, \
             tc.tile_pool(name="ex", bufs=54) as ex_pool, \
             tc.tile_pool(name="an", bufs=3) as an_pool, \
             tc.tile_pool(name="iv", bufs=4) as iv_pool, \
             tc.tile_pool(name="ob", bufs=2) as ob_pool, \
             tc.tile_pool(name="mx", bufs=2, space="PSUM") as mx, \
             tc.tile_pool(name="sc", bufs=2, space="PSUM") as scp, \
             tc.tile_pool(name="po", bufs=2, space="PSUM") as pop:

            qTs = {}   # strip qc -> [3 tiles [P, QW] f16]
            aoT = {}   # strip qc -> [3 tiles [P, QW] f16]
            ans = {}   # (qc, qt) -> normalized ao_nat tile
            exs = {}   # (kc, m) -> (ex0, ex1)
            obs = {}   # strip qc -> packed output staging tile
            pvb = {}   # live projB state per s

            def rope(pp, s):
                """psum QKV chunk [P, F] -> rotated fp16 sbuf tile.
                Act copies psum->sbuf (frees the bank), DVE does the muls,
                Pool the final add."""
                pc = rp_pool.tile([P, F], F32, tag="pc", name="pc")
                nc.scalar.copy(pc[:], pp[:])
                cs = csn[:, s * 2 * F:s * 2 * F + F]
                sn = csn[:, s * 2 * F + F:s * 2 * F + 2 * F]
                p3 = pc.rearrange("p (h d) -> p h d", h=NH)
                s3 = sn.rearrange("p (h d) -> p h d", h=NH)
                t1 = rp_pool.tile([P, F], F32, tag="t1", name="t1")
                nc.vector.tensor_mul(t1[:], pc[:], cs[:])
                t2 = rp_pool.tile([P, F], F32, tag="t2", name="t2")
                t23 = t2.rearrange("p (h d) -> p h d", h=NH)
                nc.vector.tensor_mul(t23[:, :, 0:32], p3[:, :, 32:64],
                                     s3[:, :, 0:32])
                nc.vector.tensor_mul(t23[:, :, 32:64], p3[:, :, 0:32],
                                     s3[:, :, 32:64])
                qr = qr_pool.tile([P, F], F16, tag="qr", name="qr")
                nc.gpsimd.tensor_add(qr[:], t1[:], t2[:])
                return qr

            def emit_projA(s):
                qc = s // NQ
                if s % 4 == 0:
                    qTs[qc] = qTs_pool.tile([P, MC * QW], F16, tag="qTs",
                                            name="qTs")
                pq = mx.tile([P, F], F32, tag="mx", name="pq")
                for f in range(FC):
                    nc.tensor.matmul(pq[:], xslice(f, s), wslice(0, f),
                                     start=(f == 0), stop=(f == FC - 1))
                pk = mx.tile([P, F], F32, tag="mx", name="pk")
                for f in range(FC):
                    nc.tensor.matmul(pk[:], xslice(f, s), wslice(1, f),
                                     start=(f == 0), stop=(f == FC - 1))
                qr = rope(pq, s)
                return (s, pk, qr)

            def emit_projB(state):
                s, pk, qr = state
                qc, scol = s // NQ, (s % 4) * P
                pv_ = mx.tile([P, F], F32, tag="mx", name="pv")
                for f in range(FC):
                    nc.tensor.matmul(pv_[:], xslice(f, s), wslice(2, f),
                                     start=(f == 0), stop=(f == FC - 1))
                kr = rope(pk, s)
                tq = mx.tile([P, F], F16, tag="mx", name="tpq")
                for m in range(MC):
                    nc.tensor.transpose(tq[:, m * P:(m + 1) * P],
                                        qr[:, m * P:(m + 1) * P], eye_sb[:])
                qd = qTs[qc].rearrange("p (m c) -> p m c", m=MC)
                nc.vector.tensor_copy(qd[:, :, scol:scol + P],
                                      tq.rearrange("p (m c) -> p m c", m=MC))
                tk = mx.tile([P, F], F16, tag="mx", name="tpk")
                for m in range(MC):
                    nc.tensor.transpose(tk[:, m * P:(m + 1) * P],
                                        kr[:, m * P:(m + 1) * P], eye_sb[:])
                kd = kTs.rearrange("p (m c) -> p m c", m=MC)
                nc.vector.tensor_copy(kd[:, :, s * P:(s + 1) * P],
                                      tk.rearrange("p (m c) -> p m c", m=MC))
                v3 = vo[s].rearrange("p (h e) -> p h e", h=NH)
                p3 = pv_.rearrange("p (h d) -> p h d", h=NH)
                nc.vector.tensor_copy(v3[:, :, 0:64], p3[:])

            def emit_scores(qc, kc, m):
                """head-pair m scores for block kc of strip qc + merged exp."""
                q0, k0 = qc * QW, kc * P
                off = max(0, k0 - q0)
                sp = scp.tile([P, 2 * QW], F32, tag="sc", name="sp")
                for par in range(2):
                    b = par * QW
                    d0 = 64 * par
                    lhsT = kTs[d0:d0 + 64, m * S + k0:m * S + k0 + P]
                    rhs = qTs[qc][:, m * QW:(m + 1) * QW]
                    if k0 >= q0:  # diagonal block: fold mask into PE
                        nc.tensor.matmul(sp[:, b + off:b + off + P],
                                         lhsT, rhs[d0:d0 + 64, off:off + P],
                                         start=True, stop=False)
                        nc.tensor.matmul(sp[:, b + off:b + off + P],
                                         eye_sb[:], tri_sb[:],
                                         start=False, stop=True)
                        if off + P < QW:
                            nc.tensor.matmul(sp[:, b + off + P:b + QW],
                                             lhsT, rhs[d0:d0 + 64, off + P:QW],
                                             start=True, stop=True)
                    else:
                        nc.tensor.matmul(sp[:, b:b + QW],
                                         lhsT, rhs[d0:d0 + 64, :],
                                         start=True, stop=True)
                ex = ex_pool.tile([P, 2 * QW], F8, tag="ex", name="ex")
                sp3 = sp.rearrange("p (r c) -> p r c", r=2)
                ex3 = ex.rearrange("p (r c) -> p r c", r=2)
                nc.scalar.activation(ex3[:, :, off:QW], sp3[:, :, off:QW],
                                     AF.Exp, scale=0.125)
                exs[(kc, m)] = ex

            def emit_pv(qc, qt):
                """qt-serial PV: accumulate po over all kc of the strip."""
                t = NQ * qc + qt
                po = pop.tile([P, NH * 65], F32, tag="po", name="po")
                for kc in range(t + 1):
                    for m in range(MC):
                        ex = exs[(kc, m)]
                        for par in range(2):
                            h = 2 * m + par
                            lhsT = ex[:, par * QW + qt * P:
                                      par * QW + (qt + 1) * P]
                            nc.tensor.matmul(po[:, h * 65:h * 65 + 65],
                                             lhsT, vo[kc][:, h * 65:h * 65 + 65],
                                             start=(kc == 0), stop=(kc == t))
                return po

            def emit_norm(qc, qt, po):
                """copy po to SBUF, reciprocal, per-head scale on Pool."""
                pz = iv_pool.tile([P, NH * 65], F32, tag="pz", name="pz")
                nc.vector.tensor_copy(pz[:], po[:])
                pz3 = pz.rearrange("p (h e) -> p h e", h=NH)
                inv = iv_pool.tile([P, NH], F32, tag="inv", name="inv")
                with nc.allow_low_precision(reason="softmax sums"):
                    nc.vector.reciprocal(inv[:], pz3[:, :, 64])
                an = an_pool.tile([P, F], F16, tag="an", name="an")
                for h in range(NH):
                    nc.gpsimd.tensor_scalar_mul(an[:, h * D:(h + 1) * D],
                                                pz3[:, h, 0:D],
                                                inv[:, h:h + 1])
                ans[(qc, qt)] = an

            def emit_oproj(qc, qt):
                """deferred PE fill: transpose ao_nat -> aoT, fin, store."""
                if qt == 0:
                    aoT[qc] = ao_pool.tile([P, MC * QW], F16, tag="aoT",
                                           name="aoT")
                an = ans.pop((qc, qt))
                ta = mx.tile([P, F], F16, tag="mx", name="tpa")
                for m in range(MC):
                    nc.tensor.transpose(ta[:, m * P:(m + 1) * P],
                                        an[:, m * P:(m + 1) * P], eye_sb[:])
                ad = aoT[qc].rearrange("p (m c) -> p m c", m=MC)
                nc.vector.tensor_copy(ad[:, :, qt * P:(qt + 1) * P],
                                      ta.rearrange("p (m c) -> p m c", m=MC))
                ob = ob_pool.tile([P, HID], F32, tag="ob", name="ob")
                for half in range(2):
                    c0 = half * F
                    fin = mx.tile([P, F], F32, tag="mx", name="fin")
                    for m in range(MC):
                        nc.tensor.matmul(fin[:],
                                         aoT[qc][:, m * QW + qt * P:
                                                 m * QW + (qt + 1) * P],
                                         woT[:, m * HID + c0:m * HID + c0 + F],
                                         start=(m == 0), stop=(m == MC - 1))
                    nc.vector.tensor_copy(ob[:, c0:c0 + F], fin[:])
                s0 = (NQ * qc + qt) * P
                nc.sync.dma_start(out_d[s0:s0 + P, :], ob[:])

            # ---- emission schedule ----
            fills = deque()
            for s in range(4, SC):
                fills.append(("A", s))
                fills.append(("B", s))

            def pop_fill():
                if not fills:
                    return
                kind, a = fills.popleft()
                if kind == "A":
                    pvb[a] = emit_projA(a)
                elif kind == "B":
                    emit_projB(pvb.pop(a))
                else:
                    emit_oproj(*a)

            for s in range(4):
                st = emit_projA(s)
                emit_projB(st)
            for c in range(1, NQ):
                nc.sync.dma_start(xTp[:, c * (XW // 4):(c + 1) * (XW // 4)],
                                  xTp_d[:, c * (XW // 4):(c + 1) * (XW // 4)])
                nc.sync.dma_start(csn[:, c * (CW // 4):(c + 1) * (CW // 4)],
                                  csn_d[:, c * (CW // 4):(c + 1) * (CW // 4)])

            for qc in range(NQ):
                exs.clear()
                last = NQ * qc + 3
                for kc in range(last + 1):
                    pop_fill()
                    if qc > 0:
                        pop_fill()
                    for m in range(MC):
                        emit_scores(qc, kc, m)
                    if kc >= NQ * qc:
                        qt = kc - NQ * qc
                        po = emit_pv(qc, qt)
                        emit_norm(qc, qt, po)
                        fills.append(("O", (qc, qt)))
            while fills:
                pop_fill()
    nc.compile()
    return nc


def _rope_tables():
    inv_freq = 1.0 / (ROPE_THETA ** (np.arange(0, D, 2, dtype=np.float32) / D))
    t = np.arange(S, dtype=np.float32)
    freqs = np.outer(t, inv_freq)                       # [S, 32]
    emb = np.concatenate([freqs, freqs], axis=-1)       # [S, 64]
    cos = np.cos(emb).astype(np.float32)
    sin = np.sin(emb).astype(np.float32)
    sin_signed = sin.copy()
    sin_signed[:, 0:32] *= -1.0                         # fold rotate_half sign
    cos6 = np.tile(cos, (1, NH))                        # [S, 384]
    sin6 = np.tile(sin_signed, (1, NH))
    # pack [cos | sin] per s-tile: [128, 16*768]
    both = np.concatenate(
        [cos6.reshape(SC, P, F), sin6.reshape(SC, P, F)], axis=2)  # [16,128,768]
    return np.ascontiguousarray(
        both.transpose(1, 0, 2).reshape(P, CW)).astype(np.float16)


_STATE = {}


def _get_program():
    if "nc" not in _STATE:
        _STATE["nc"] = build_program()
    return _STATE["nc"]


def _pack_x(xT):
    """[768, 2048] -> [128, 12288] with cols (c, f, 512)."""
    v = xT.reshape(FC, P, NQ, QW)               # f, p, c, col
    return np.ascontiguousarray(
        v.transpose(1, 2, 0, 3).reshape(P, XW))  # p, (c f col)


def _pack_w(Wq, Wk, Wv, cols):
    """3x [768, 384] (transposed slices) -> [128, 6912] cols (f, kind, 384)."""
    ws = [np.asarray(W[cols, :].T, dtype=np.float32).reshape(FC, P, F)
          for W in (Wq, Wk, Wv)]
    stk = np.stack(ws, axis=2)                   # f, p, kind, 384
    return np.ascontiguousarray(stk.transpose(1, 0, 2, 3).reshape(P, WW))


def _make_in_maps(hidden_states, Wq, Wk, Wv, Wo):
    hs = np.asarray(hidden_states, dtype=np.float32)
    Wq = np.asarray(Wq, dtype=np.float32)
    Wk = np.asarray(Wk, dtype=np.float32)
    Wv = np.asarray(Wv, dtype=np.float32)
    Wo = np.asarray(Wo, dtype=np.float32)

    csn = _rope_tables()
    trineg = (NEG * np.tril(np.ones((P, P), dtype=np.float32), -1)
              ).astype(np.float16)
    eye = np.eye(P, dtype=np.float16)

    in_maps = []
    for c in range(N_CORES):
        b, g = c // 2, c % 2
        cols = slice(g * F, (g + 1) * F)
        woT = np.asarray(Wo[:, cols].T, dtype=np.float32)    # [384, 768]
        woTp = np.ascontiguousarray(
            woT.reshape(MC, P, HID).transpose(1, 0, 2).reshape(P, MC * HID))
        in_maps.append({
            "xTp": _pack_x(hs[b].T).astype(np.float16),
            "wp": _pack_w(Wq, Wk, Wv, cols).astype(np.float16),
            "woT": woTp.astype(np.float16),
            "csn": csn,
            "trineg": trineg,
            "eye": eye,
        })
    return in_maps


def run(hidden_states, Wq, Wk, Wv, Wo, trace=False, **trace_kw):
    nc = _get_program()
    in_maps = _make_in_maps(hidden_states, Wq, Wk, Wv, Wo)
    res = run_bass_kernel_spmd(nc, in_maps, core_ids=list(range(N_CORES)),
                               trace=trace, **trace_kw)
    B = 4
    out = np.empty((B, S, HID), dtype=np.float32)
    for b in range(B):
        out[b] = res.results[2 * b]["out"] + res.results[2 * b + 1]["out"]
    return out, res


def kernel(hidden_states, Wq, Wk, Wv, Wo):
    out, _ = run(hidden_states, Wq, Wk, Wv, Wo,
                 trace=bool(int(os.environ.get("KERNEL_TRACE", "0"))))
    return out
